# revision 1
# baseline (speedup 1.0000x reference)
"""GAT (graph attention) Bass kernel for Trainium2, 8-core SPMD — v2.

Strategy: receiver-per-partition windows + chunked indirect-DMA gathers.

Host sorts active nodes by degree and packs them 128 per window (one SBUF
partition per receiver). Windows are dealt round-robin to the 8 cores so
every core runs one shared instruction stream; the per-window slot count
K[w] (edge blocks of 128 slots) is the max over the 8 cores' windows.

Device kernel, per core:
  phase A: tab[n] = [h(64) | s1(4) | s2(4) | pad] fp16 256B rows, written
           block-permuted so the stores are fully contiguous (the gather
           indices absorb the permutation); one sentinel row at npad with
           h=0, s1=-100 (=> pad slots exp to exactly 0 in fp16).
  phase B: per chunk of windows, indirect DMAs ([128, 1] i32 offsets, one
           per 128-row block — wider offset APs gather garbage on this HW)
           fetch all sender rows plus one receiver row per window.
           Compute is pure DVE/ACT per partition: logit = s1 + s2(recv),
           leaky-relu, exp(.-3.5) (softmax-shift-invariant), then weighted
           free-axis reductions. No matmuls in phase B, no collectives.

Host scatters the staged [128, 64] window outputs back to node order.
"""

import os
import sys

import numpy as np

for _p in ("/opt/trn_rl_repo", os.path.expanduser("~/.axon_site/_ro/trn_rl_repo")):
    if os.path.isdir(_p) and _p not in sys.path:
        sys.path.insert(0, _p)

P = 128
XTILE = 1024                 # phase-A node super-tile
NBLK = XTILE // P            # 8
TCOLS = 128                  # fp16 table row = 256B
HEADS = 4
UNITS = 16
HU = HEADS * UNITS           # 64
S1OFF = HU                   # cols 64:68 = s1
S2OFF = HU + HEADS           # cols 68:72 = s2
WC = HU + 2 * HEADS          # 72 written cols
LEAKY_ALPHA = 0.2
CSHIFT = 3.5                 # global exp shift (softmax-invariant)
S1_SENTINEL = -100.0         # sentinel row: exp(leaky(s1+s2)-c) == 0 in fp16
BCAP = 128                   # max edge blocks per gather chunk
WCAP = 8                     # max windows per gather chunk
GCALL = 1                    # blocks per indirect DMA call (multi-col offset
                             # APs gather garbage on HW; keep 1)
QSPLIT = 1                   # SWDGE queues for indirect calls (2 was HW-
                             # correct but no faster; desc-gen is serial)
ABLATE = "full"              # dev-only: "phaseA" | "nocompute"
REPS = 1                     # dev-only: replicate kernel body for timing


def _perm(n):
    """Node id -> permuted table row (phase-A stores become contiguous)."""
    n = np.asarray(n)
    t, r = n // XTILE, n % XTILE
    return t * XTILE + (r % P) * NBLK + (r // P)


def _build_host_data(x, edge_index, W, att_w1, att_w2, n_cores):
    n_nodes, in_feat = x.shape
    snd = edge_index[:, 0].astype(np.int64)
    rcv = edge_index[:, 1].astype(np.int64)

    ntiles = -(-n_nodes // XTILE)
    npad = ntiles * XTILE
    sent = npad  # sentinel row index

    deg = np.bincount(rcv, minlength=n_nodes)
    active = np.nonzero(deg > 0)[0]
    order_n = active[np.argsort(deg[active], kind="stable")]

    wtot = -(-len(order_n) // P)
    nw = -(-wtot // n_cores)
    wpad = nw * n_cores
    win_nodes_g = np.full((wpad, P), -1, dtype=np.int64)
    win_nodes_g.reshape(-1)[: len(order_n)] = order_n

    deg_g = np.where(win_nodes_g >= 0, deg[win_nodes_g], 0)
    k_g = deg_g.max(axis=1)
    # per-local-window block cap: max over the n_cores interleaved windows
    K = k_g.reshape(nw, n_cores).max(axis=1).astype(np.int64)

    # chunking: greedy, <= BCAP blocks and <= WCAP windows per chunk
    chunks = []  # list of (w0, nwin)
    w = 0
    while w < nw:
        w0 = w
        blocks = 0
        while w < nw and (w - w0) < WCAP and (blocks + K[w]) <= max(BCAP, K[w]):
            blocks += K[w]
            w += 1
        chunks.append((w0, w - w0))

    # node -> (core, local w, partition)
    node_c = np.full(n_nodes, -1, dtype=np.int64)
    node_w = np.zeros(n_nodes, dtype=np.int64)
    node_p = np.zeros(n_nodes, dtype=np.int64)
    gwin = np.repeat(np.arange(wpad), P).reshape(wpad, P)
    valid = win_nodes_g >= 0
    vn = win_nodes_g[valid]
    node_c[vn] = gwin[valid] % n_cores
    node_w[vn] = gwin[valid] // n_cores
    node_p[vn] = np.tile(np.arange(P), wpad).reshape(wpad, P)[valid]

    # edge -> slot k within its receiver's run
    eorder = np.argsort(rcv, kind="stable")
    rs = rcv[eorder]
    ss = snd[eorder]
    starts = np.zeros(n_nodes + 1, dtype=np.int64)
    starts[1:] = np.cumsum(deg)
    k_e = np.arange(len(rs)) - starts[rs]
    perm_ss = _perm(ss)

    base = np.zeros(nw + 1, dtype=np.int64)
    base[1:] = np.cumsum(K)
    btot = int(base[-1])  # total sender blocks per core

    xT16 = np.zeros((in_feat, npad), dtype=np.float16)
    xT16[:, :n_nodes] = np.ascontiguousarray(x.T).astype(np.float16)

    # wcat = [W | W@A1 | W@A2] fp16  [in_feat, 72]
    A12 = np.zeros((HU, 2 * HEADS), dtype=np.float32)
    for h in range(HEADS):
        A12[h * UNITS:(h + 1) * UNITS, h] = att_w1[h, 0]
        A12[h * UNITS:(h + 1) * UNITS, HEADS + h] = att_w2[h, 0]
    wcat = np.zeros((in_feat, WC), dtype=np.float32)
    wcat[:, :HU] = W
    wcat[:, HU:] = W @ A12
    wcat16 = wcat.astype(np.float16)

    zrow = np.zeros((1, TCOLS), dtype=np.float16)
    zrow[0, S1OFF:S1OFF + HEADS] = S1_SENTINEL

    per_core = []
    win_nodes_c_all = []
    for c in range(n_cores):
        wn = win_nodes_g[c::n_cores]  # [nw, 128]
        emask = node_c[rs] == c
        er = rs[emask]
        ew = node_w[er]
        ep = node_p[er]
        ek = k_e[emask]

        sidx = np.full((btot, P), sent, dtype=np.int32)  # [block, partition]
        sidx[base[ew] + ek, ep] = perm_ss[emask].astype(np.int32)
        ridx = np.where(wn >= 0, _perm(np.maximum(wn, 0)), sent).astype(np.int32)

        per_core.append({
            "xT16": xT16,
            "wcat": wcat16,
            "zrow": zrow,
            "sidx": np.ascontiguousarray(sidx.T),   # [128, btot] i32
            "ridx": np.ascontiguousarray(ridx.T),   # [128, nw] i32
        })
        win_nodes_c_all.append(wn)

    plan = {
        "npad": npad, "ntiles": ntiles, "nw": nw,
        "K": K.tolist(), "base": base.tolist(), "btot": btot,
        "chunks": chunks, "in_feat": in_feat,
    }
    host = {"plan": plan, "win_nodes": win_nodes_c_all, "n_nodes": n_nodes}
    return host, per_core


def _build_bass(plan):
    from concourse import bacc, mybir, tile
    import concourse.bass as bass

    f16 = mybir.dt.float16
    f32 = mybir.dt.float32
    i32 = mybir.dt.int32

    npad = plan["npad"]
    ntiles = plan["ntiles"]
    nw = plan["nw"]
    K = plan["K"]
    base = plan["base"]
    btot = plan["btot"]
    chunks = plan["chunks"]
    in_feat = plan["in_feat"]

    nc = bacc.Bacc("TRN2", target_bir_lowering=False, debug=False,
                   enable_asserts=False, num_devices=1,
                   num_swdge_queues=QSPLIT)
    _gq = [0]

    def _indirect(**kw):
        r = nc.gpsimd.indirect_dma_start(**kw)
        q = _gq[0] % QSPLIT
        if q:
            r.ins.queue = f"qPoolDynamic{q}"
        _gq[0] += 1
        return r

    xT_d = nc.dram_tensor("xT16", [in_feat, npad], f16, kind="ExternalInput").ap()
    wcat_d = nc.dram_tensor("wcat", [in_feat, WC], f16, kind="ExternalInput").ap()
    zrow_d = nc.dram_tensor("zrow", [1, TCOLS], f16, kind="ExternalInput").ap()
    sidx_d = nc.dram_tensor("sidx", [P, btot], i32, kind="ExternalInput").ap()
    ridx_d = nc.dram_tensor("ridx", [P, nw], i32, kind="ExternalInput").ap()

    out_d = nc.dram_tensor("staged", [nw * P, HU], f32, kind="ExternalOutput").ap()
    tab_d = nc.dram_tensor("tab", [npad + 1, TCOLS], f16, kind="Internal").ap()

    with tile.TileContext(nc) as tc:
        with tc.tile_pool(name="consts", bufs=1) as cpool:
            wcat_sb = cpool.tile([in_feat, WC], f16, tag="wcat")
            nc.sync.dma_start(out=wcat_sb[:], in_=wcat_d[:])
            sidx_sb = cpool.tile([P, btot], i32, tag="sidx")
            nc.sync.dma_start(out=sidx_sb[:], in_=sidx_d[:])
            ridx_sb = cpool.tile([P, nw], i32, tag="ridx")
            nc.sync.dma_start(out=ridx_sb[:], in_=ridx_d[:])
            zr_sb = cpool.tile([1, TCOLS], f16, tag="zrow")
            nc.sync.dma_start(out=zr_sb[:], in_=zrow_d[:])
            nc.sync.dma_start(out=tab_d[npad:npad + 1, :], in_=zr_sb[:])
            cbias = cpool.tile([P, 1], f32, tag="cbias")
            nc.gpsimd.memset(cbias[:], -CSHIFT)
            zpad = cpool.tile([P, NBLK * (TCOLS - WC)], f16, tag="zpad")
            nc.gpsimd.memset(zpad[:], 0.0)

            # ---- phase A: node table ----
            with tc.tile_pool(name="pa_x", bufs=3) as pax, \
                 tc.tile_pool(name="pa_ps", bufs=3, space="PSUM") as paps, \
                 tc.tile_pool(name="pa_hs", bufs=3) as pahs:
              for _rep in range(REPS):
                for t in range(ntiles):
                    xt = pax.tile([in_feat, XTILE], f16, tag="xt")
                    nc.sync.dma_start(
                        out=xt[:], in_=xT_d[:, t * XTILE:(t + 1) * XTILE])
                    hst = pahs.tile([P, NBLK * TCOLS], f16, tag="hst")
                    hst3 = hst[:].rearrange("p (i c) -> p i c", c=TCOLS)
                    nc.vector.tensor_copy(
                        out=hst3[:, :, WC:TCOLS],
                        in_=zpad[:].rearrange("p (i c) -> p i c",
                                              c=TCOLS - WC))
                    half = NBLK // 2
                    for g in range(2):
                        ps = paps.tile([P, half * WC], f32, tag="ps")
                        for i in range(half):
                            b = g * half + i
                            nc.tensor.matmul(
                                out=ps[:, i * WC:(i + 1) * WC],
                                lhsT=xt[:, b * P:(b + 1) * P],
                                rhs=wcat_sb[:], start=True, stop=True)
                        nc.vector.tensor_copy(
                            out=hst3[:, g * half:(g + 1) * half, 0:WC],
                            in_=ps[:].rearrange("p (i c) -> p i c", c=WC))
                    nc.sync.dma_start(
                        out=tab_d[t * XTILE:(t + 1) * XTILE, :].rearrange(
                            "(p i) c -> p i c", p=P),
                        in_=hst3)

            # ---- phase B: windows ----
            if ABLATE != "phaseA":
              with tc.tile_pool(name="pb_hs", bufs=2) as pbh, \
                   tc.tile_pool(name="pb_rg", bufs=2) as pbr, \
                   tc.tile_pool(name="pb_w", bufs=3) as pbw, \
                   tc.tile_pool(name="pb_o", bufs=2) as pbo:
                for _rep in range(REPS):
                  for (w0, nwin) in chunks:
                    nb = sum(K[w0:w0 + nwin])
                    if nb == 0:
                        continue
                    hs = pbh.tile([P, nb * TCOLS], f16, tag="hs")
                    hs3 = hs[:].rearrange("p (j c) -> p j c", c=TCOLS)
                    for b0 in range(0, nb, GCALL):
                        b1 = min(b0 + GCALL, nb)
                        _indirect(
                            out=hs3[:, b0:b1, :] if b1 - b0 > 1
                            else hs3[:, b0, :],
                            out_offset=None, in_=tab_d[:],
                            in_offset=bass.IndirectOffsetOnAxis(
                                ap=sidx_sb[:, base[w0] + b0:base[w0] + b1],
                                axis=0))
                    rg = pbr.tile([P, nwin * TCOLS], f16, tag="rg")
                    rg3 = rg[:].rearrange("p (j c) -> p j c", c=TCOLS)
                    for b0 in range(0, nwin, GCALL):
                        b1 = min(b0 + GCALL, nwin)
                        _indirect(
                            out=rg3[:, b0:b1, :] if b1 - b0 > 1
                            else rg3[:, b0, :],
                            out_offset=None, in_=tab_d[:],
                            in_offset=bass.IndirectOffsetOnAxis(
                                ap=ridx_sb[:, w0 + b0:w0 + b1], axis=0))
                    if ABLATE == "nocompute":
                        continue

                    osb_c = pbo.tile([P, nwin * HU], f32, tag="osb")
                    osb3 = osb_c[:].rearrange("p (i c) -> p i c", c=HU)
                    off = 0
                    for i in range(nwin):
                        w = w0 + i
                        k = K[w]
                        if k == 0:
                            nc.gpsimd.memset(osb3[:, i, :], 0.0)
                            continue
                        lg = pbw.tile([P, k * HEADS], f16, tag="lg")
                        lg3 = lg[:].rearrange("p (j h) -> p j h", h=HEADS)
                        nc.vector.tensor_tensor(
                            out=lg3,
                            in0=hs3[:, off:off + k, S1OFF:S1OFF + HEADS],
                            in1=rg3[:, i, S2OFF:S2OFF + HEADS].unsqueeze(
                                1).broadcast_to([P, k, HEADS]),
                            op=mybir.AluOpType.add)
                        neg = pbw.tile([P, k * HEADS], f16, tag="neg")
                        nc.vector.tensor_scalar(
                            out=neg[:], in0=lg[:], scalar1=0.0,
                            scalar2=LEAKY_ALPHA, op0=mybir.AluOpType.min,
                            op1=mybir.AluOpType.mult)
                        lr = pbw.tile([P, k * HEADS], f16, tag="lr")
                        nc.vector.scalar_tensor_tensor(
                            out=lr[:], in0=lg[:], scalar=0.0, in1=neg[:],
                            op0=mybir.AluOpType.max, op1=mybir.AluOpType.add)
                        expo = pbw.tile([P, k * HEADS], f16, tag="expo")
                        nc.scalar.activation(
                            out=expo[:], in_=lr[:],
                            func=mybir.ActivationFunctionType.Exp,
                            bias=cbias[:])
                        ex3 = expo[:].rearrange("p (j h) -> p j h", h=HEADS)
                        rhs = pbw.tile([P, k * HU], f16, tag="rhs")
                        nc.vector.tensor_tensor(
                            out=rhs[:].rearrange("p (j h u) -> p j h u",
                                                 h=HEADS, u=UNITS),
                            in0=hs3[:, off:off + k, 0:HU].rearrange(
                                "p j (h u) -> p j h u", u=UNITS),
                            in1=ex3.unsqueeze(3).broadcast_to(
                                [P, k, HEADS, UNITS]),
                            op=mybir.AluOpType.mult)
                        den = pbw.tile([P, HEADS], f32, tag="den")
                        nc.vector.tensor_reduce(
                            out=den[:],
                            in_=expo[:].rearrange("p (j h) -> p h j", h=HEADS),
                            axis=mybir.AxisListType.X, op=mybir.AluOpType.add)
                        num = pbw.tile([P, HU], f32, tag="num")
                        nc.vector.tensor_reduce(
                            out=num[:],
                            in_=rhs[:].rearrange("p (j c) -> p c j", c=HU),
                            axis=mybir.AxisListType.X, op=mybir.AluOpType.add)
                        den2 = pbw.tile([P, HEADS], f32, tag="den2")
                        nc.vector.tensor_scalar_add(
                            out=den2[:], in0=den[:], scalar1=1e-30)
                        rec = pbw.tile([P, HEADS], f32, tag="rec")
                        nc.vector.reciprocal(out=rec[:], in_=den2[:])
                        nc.vector.tensor_tensor(
                            out=osb3[:, i, :].rearrange("p (h u) -> p h u",
                                                        u=UNITS),
                            in0=num[:].rearrange("p (h u) -> p h u", u=UNITS),
                            in1=rec[:].unsqueeze(2).broadcast_to(
                                [P, HEADS, UNITS]),
                            op=mybir.AluOpType.mult)
                        off += k
                    nc.sync.dma_start(
                        out=out_d[w0 * P:(w0 + nwin) * P, :].rearrange(
                            "(i p) c -> p i c", p=P),
                        in_=osb3)

    nc.compile()
    return nc


def _run(nc, per_core, n_cores):
    from concourse import bass_utils

    want_trace = bool(os.environ.get("GAT_TRACE"))
    res = bass_utils.run_bass_kernel_spmd(
        nc, per_core, core_ids=list(range(n_cores)), trace=want_trace)
    return res


def _unshard(host, results, n_cores):
    n_nodes = host["n_nodes"]
    out = np.zeros((n_nodes, HU), dtype=np.float32)
    for c in range(n_cores):
        staged = results[c]["staged"]  # [nw*128, 64]
        wn = host["win_nodes"][c]      # [nw, 128]
        valid = wn >= 0
        out[wn[valid]] = staged.reshape(wn.shape[0], P, HU)[valid]
    return out


def kernel(x, edge_index, W, att_w1, att_w2, n_cores=8, _return_results=False):
    x = np.asarray(x)
    edge_index = np.asarray(edge_index)
    W = np.asarray(W).astype(np.float32)
    att_w1 = np.asarray(att_w1).astype(np.float32)
    att_w2 = np.asarray(att_w2).astype(np.float32)

    host, per_core = _build_host_data(x, edge_index, W, att_w1, att_w2, n_cores)
    nc = _build_bass(host["plan"])
    res = _run(nc, per_core, n_cores)
    out = _unshard(host, res.results, n_cores)
    if _return_results:
        return out, res
    return out



# revision 4
# speedup vs baseline: 1.5258x; 1.5258x over previous
"""GAT (graph attention) Bass kernel for Trainium2, 8-core SPMD — v3.

Strategy: receiver-per-partition windows + batched dma_gather.

v2 issued one indirect DMA per 128-row block (~900 calls/core x ~1us
SWDGE fixed cost = the whole phase-B budget). v3 packs each chunk's
receiver+sender rows into ONE InstDMAGatherAnt call (14 calls/core).

dma_gather indices are int16 (max 32767), so the node table stores
PAIRS of h-vectors: row j = [h(2j) | h(2j+1)] fp16 (256B), 25088 pairs
+ 1 zero sentinel < 32767. Attention scores are no longer precomputed
in the table; phase B computes s1/s2 for BOTH pair halves from the
gathered row (a1cat/a2cat elementwise + 16-group reduce) and a host-
built mask stream adds 0 / -100 per (slot, half): the wrong half and
pad slots exp() to exactly 0 in fp16, so numerator (128-col) and
denominator (8-col) reductions just fold A+B halves at the end.

Per core:
  phase A: tab2[j] = [h(2j)(64) | h(2j+1)(64)] fp16, written via the
           same block-permuted contiguous stores as v2 (gather indices
           absorb the permutation); one zero sentinel row at `pairs`.
  phase B: per chunk (<=8 windows, <=64 sender blocks): one dma_gather
           fetches nwin receiver blocks + nb sender blocks (128 rows
           each) from tab2; DVE/ACT compute per window; output DMA.

Host scatters the staged [128, 64] window outputs back to node order.
"""

import os
import sys

import numpy as np

for _p in ("/opt/trn_rl_repo", os.path.expanduser("~/.axon_site/_ro/trn_rl_repo")):
    if os.path.isdir(_p) and _p not in sys.path:
        sys.path.insert(0, _p)

P = 128
XTILE = 1024                 # phase-A node super-tile
NBLK = XTILE // P            # 8
HEADS = 4
UNITS = 16
HU = HEADS * UNITS           # 64
PAIRC = 2 * HU               # 128 fp16 cols = 256B pair row
LEAKY_ALPHA = 0.2
CSHIFT = 3.5                 # global exp shift (softmax-invariant)
PEN = -100.0                 # per-(slot,half) penalty => exp==0 in fp16
BCAP = 64                    # max sender blocks per gather chunk
WCAP = 8                     # max windows per gather chunk
ABLATE = "full"              # dev-only: "phaseA" | "nocompute"
REPS = 1                     # dev-only: replicate kernel body for timing


def _pair(n):
    """Node id -> (pair row, half) in the block-permuted pair table."""
    n = np.asarray(n)
    t, r = n // XTILE, n % XTILE
    p, i = r % P, r // P
    return t * (XTILE // 2) + p * (NBLK // 2) + i // 2, i % 2


def _build_host_data(x, edge_index, W, att_w1, att_w2, n_cores):
    n_nodes, in_feat = x.shape
    snd = edge_index[:, 0].astype(np.int64)
    rcv = edge_index[:, 1].astype(np.int64)

    ntiles = -(-n_nodes // XTILE)
    npad = ntiles * XTILE
    pairs = npad // 2
    sent = pairs  # sentinel pair row (zeros)

    deg = np.bincount(rcv, minlength=n_nodes)
    active = np.nonzero(deg > 0)[0]
    order_n = active[np.argsort(deg[active], kind="stable")]

    wtot = -(-len(order_n) // P)
    nw = -(-wtot // n_cores)
    wpad = nw * n_cores
    win_nodes_g = np.full((wpad, P), -1, dtype=np.int64)
    win_nodes_g.reshape(-1)[: len(order_n)] = order_n

    deg_g = np.where(win_nodes_g >= 0, deg[win_nodes_g], 0)
    k_g = deg_g.max(axis=1)
    # per-local-window block cap: max over the n_cores interleaved windows
    K = k_g.reshape(nw, n_cores).max(axis=1).astype(np.int64)

    # chunking: greedy, <= BCAP sender blocks and <= WCAP windows per chunk
    chunks = []  # list of (w0, nwin)
    w = 0
    while w < nw:
        w0 = w
        blocks = 0
        while w < nw and (w - w0) < WCAP and (blocks + K[w]) <= max(BCAP, K[w]):
            blocks += K[w]
            w += 1
        chunks.append((w0, w - w0))

    base = np.zeros(nw + 1, dtype=np.int64)
    base[1:] = np.cumsum(K)
    btot = int(base[-1])
    TB = nw + btot  # total stream blocks per core

    # stream columns: per chunk [recv blocks | sender blocks]
    rcol = np.zeros(nw, dtype=np.int64)   # stream block of window w's recv row
    scol = np.zeros(nw, dtype=np.int64)   # stream block of window w's 1st sender
    for (w0, nwin) in chunks:
        S = w0 + base[w0]
        for i in range(nwin):
            rcol[w0 + i] = S + i
            scol[w0 + i] = S + nwin + (base[w0 + i] - base[w0])

    # node -> (core, local w, partition)
    node_c = np.full(n_nodes, -1, dtype=np.int64)
    node_w = np.zeros(n_nodes, dtype=np.int64)
    node_p = np.zeros(n_nodes, dtype=np.int64)
    gwin = np.repeat(np.arange(wpad), P).reshape(wpad, P)
    valid = win_nodes_g >= 0
    vn = win_nodes_g[valid]
    node_c[vn] = gwin[valid] % n_cores
    node_w[vn] = gwin[valid] // n_cores
    node_p[vn] = np.tile(np.arange(P), wpad).reshape(wpad, P)[valid]

    # edge -> slot k within its receiver's run
    eorder = np.argsort(rcv, kind="stable")
    rs = rcv[eorder]
    ss = snd[eorder]
    starts = np.zeros(n_nodes + 1, dtype=np.int64)
    starts[1:] = np.cumsum(deg)
    k_e = np.arange(len(rs)) - starts[rs]
    pair_ss, half_ss = _pair(ss)

    xT16 = np.zeros((in_feat, npad), dtype=np.float16)
    xT16[:, :n_nodes] = np.ascontiguousarray(x.T).astype(np.float16)

    w16 = W.astype(np.float16)  # [in_feat, 64]

    # a1cat/a2cat: [128] = per-pair-col attention vector, replicated to 128
    # partitions -> aa [128, 256] = [a1cat | a2cat]
    a1 = att_w1.reshape(HEADS, UNITS).astype(np.float16)  # [4, 16]
    a2 = att_w2.reshape(HEADS, UNITS).astype(np.float16)
    a1cat = np.concatenate([a1.reshape(-1), a1.reshape(-1)])  # [128]
    a2cat = np.concatenate([a2.reshape(-1), a2.reshape(-1)])
    aa = np.tile(np.concatenate([a1cat, a2cat])[None, :], (P, 1))  # [128, 256]

    per_core = []
    win_nodes_c_all = []
    for c in range(n_cores):
        wn = win_nodes_g[c::n_cores]  # [nw, 128]
        emask = node_c[rs] == c
        er = rs[emask]
        ew = node_w[er]
        ep = node_p[er]
        ek = k_e[emask]

        # flat stream position j = B*128 + p
        idxf = np.full(TB * P, sent, dtype=np.int32)
        half = np.zeros(TB * P, dtype=np.int64)
        kind = np.zeros(TB * P, dtype=np.int8)  # 0=pad, 1=sender, 2=recv

        j_e = (scol[ew] + ek) * P + ep
        idxf[j_e] = pair_ss[emask]
        half[j_e] = half_ss[emask]
        kind[j_e] = 1

        rvalid = wn >= 0
        pr, hr = _pair(np.maximum(wn, 0))
        j_r = rcol[:, None] * P + np.arange(P)[None, :]  # [nw, 128]
        idxf[j_r.reshape(-1)] = np.where(rvalid, pr, sent).reshape(-1)
        half[j_r.reshape(-1)] = np.where(rvalid, hr, 0).reshape(-1)
        kind[j_r.reshape(-1)] = np.where(rvalid, 2, 0).reshape(-1)

        # mask stream [TB*P, 8]: senders get {0, PEN} penalties per half,
        # receivers get {1, 0} selector bits per half, pads get PEN.
        mask = np.zeros((TB * P, 8), dtype=np.float16)
        isA = half == 0
        s = kind == 1
        mask[s, 0:4] = np.where(isA[s, None], 0.0, PEN)
        mask[s, 4:8] = np.where(isA[s, None], PEN, 0.0)
        r = kind == 2
        mask[r, 0:4] = np.where(isA[r, None], 1.0, 0.0)
        mask[r, 4:8] = np.where(isA[r, None], 0.0, 1.0)
        pad = kind == 0
        mask[pad, :] = PEN

        # idx16: wrapped-16 layout [16, TB*8] replicated to [128, TB*8]
        idxw = idxf.astype(np.int16).reshape(TB * 8, 16).T  # [16, TB*8]
        idx16 = np.ascontiguousarray(np.tile(idxw, (8, 1)))  # [128, TB*8]

        mask16 = np.ascontiguousarray(
            mask.reshape(TB, P, 8).transpose(1, 0, 2).reshape(P, TB * 8))

        per_core.append({
            "xT16": xT16,
            "w16": w16,
            "aa": aa,
            "idx16": idx16,
            "mask16": mask16,
        })
        win_nodes_c_all.append(wn)

    plan = {
        "npad": npad, "ntiles": ntiles, "nw": nw, "pairs": pairs,
        "K": K.tolist(), "base": base.tolist(), "btot": btot, "TB": TB,
        "chunks": chunks, "in_feat": in_feat,
    }
    host = {"plan": plan, "win_nodes": win_nodes_c_all, "n_nodes": n_nodes}
    return host, per_core


def _build_bass(plan):
    from concourse import bacc, mybir, tile
    import concourse.bass as bass

    f16 = mybir.dt.float16
    f32 = mybir.dt.float32
    i16 = mybir.dt.int16

    npad = plan["npad"]
    ntiles = plan["ntiles"]
    nw = plan["nw"]
    pairs = plan["pairs"]
    K = plan["K"]
    base = plan["base"]
    TB = plan["TB"]
    chunks = plan["chunks"]
    in_feat = plan["in_feat"]

    nc = bacc.Bacc("TRN2", target_bir_lowering=False, debug=False,
                   enable_asserts=False, num_devices=1)

    xT_d = nc.dram_tensor("xT16", [in_feat, npad], f16, kind="ExternalInput").ap()
    w_d = nc.dram_tensor("w16", [in_feat, HU], f16, kind="ExternalInput").ap()
    aa_d = nc.dram_tensor("aa", [P, 2 * PAIRC], f16, kind="ExternalInput").ap()
    idx_d = nc.dram_tensor("idx16", [P, TB * 8], i16, kind="ExternalInput").ap()
    mask_d = nc.dram_tensor("mask16", [P, TB * 8], f16, kind="ExternalInput").ap()

    out_d = nc.dram_tensor("staged", [nw * P, HU], f32, kind="ExternalOutput").ap()
    tab_d = nc.dram_tensor("tab2", [pairs + 1, PAIRC], f16, kind="Internal").ap()

    with tile.TileContext(nc) as tc:
        with tc.tile_pool(name="consts", bufs=1) as cpool:
            w_sb = cpool.tile([in_feat, HU], f16, tag="w16")
            nc.sync.dma_start(out=w_sb[:], in_=w_d[:])
            aa_sb = cpool.tile([P, 2 * PAIRC], f16, tag="aa")
            nc.sync.dma_start(out=aa_sb[:], in_=aa_d[:])
            idx_sb = cpool.tile([P, TB * 8], i16, tag="idx16")
            nc.sync.dma_start(out=idx_sb[:], in_=idx_d[:])
            mask_sb = cpool.tile([P, TB * 8], f16, tag="mask16")
            nc.sync.dma_start(out=mask_sb[:], in_=mask_d[:])
            zrow = cpool.tile([1, PAIRC], f16, tag="zrow")
            nc.gpsimd.memset(zrow[:], 0.0)
            nc.sync.dma_start(out=tab_d[pairs:pairs + 1, :], in_=zrow[:])
            cbias = cpool.tile([P, 1], f32, tag="cbias")
            nc.gpsimd.memset(cbias[:], -CSHIFT)

            # ---- phase A: pair table tab2[t*512 + p*4 + i] ----
            with tc.tile_pool(name="pa_x", bufs=3) as pax, \
                 tc.tile_pool(name="pa_ps", bufs=4, space="PSUM") as paps, \
                 tc.tile_pool(name="pa_hs", bufs=3) as pahs:
              for _rep in range(REPS):
                for t in range(ntiles):
                    xt = pax.tile([in_feat, XTILE], f16, tag="xt")
                    nc.sync.dma_start(
                        out=xt[:], in_=xT_d[:, t * XTILE:(t + 1) * XTILE])
                    hst = pahs.tile([P, NBLK * HU], f16, tag="hst")
                    half = NBLK // 2
                    for g in range(2):
                        ps = paps.tile([P, half * HU], f32, tag="ps")
                        for i in range(half):
                            b = g * half + i
                            nc.tensor.matmul(
                                out=ps[:, i * HU:(i + 1) * HU],
                                lhsT=xt[:, b * P:(b + 1) * P],
                                rhs=w_sb[:], start=True, stop=True)
                        nc.vector.tensor_copy(
                            out=hst[:, g * half * HU:(g + 1) * half * HU],
                            in_=ps[:])
                    nc.sync.dma_start(
                        out=tab_d[t * (XTILE // 2):(t + 1) * (XTILE // 2),
                                  :].rearrange("(p i) c -> p i c", p=P),
                        in_=hst[:].rearrange("p (i c) -> p i c", c=PAIRC))

            # ---- phase B: chunked gather + per-window attention ----
            if ABLATE != "phaseA":
              with tc.tile_pool(name="pb_hs", bufs=2) as pbh, \
                   tc.tile_pool(name="pb_w", bufs=3) as pbw, \
                   tc.tile_pool(name="pb_o", bufs=2) as pbo:
                for _rep in range(REPS):
                  for (w0, nwin) in chunks:
                    nb = sum(K[w0:w0 + nwin])
                    S = w0 + base[w0]
                    nbt = nwin + nb
                    hs = pbh.tile([P, nbt * PAIRC], f16, tag="hs")
                    hs3 = hs[:].rearrange("p (j c) -> p j c", c=PAIRC)
                    nc.gpsimd.dma_gather(
                        out_ap=hs3,
                        in_ap=tab_d[:],
                        idxs_ap=idx_sb[:, S * 8:(S + nbt) * 8],
                        num_idxs=nbt * P,
                        num_idxs_reg=nbt * P,
                        elem_size=PAIRC,
                        single_packet=False)
                    if ABLATE == "nocompute":
                        continue
                    m3 = mask_sb[:, S * 8:(S + nbt) * 8].rearrange(
                        "p (j q) -> p j q", q=8)

                    osb_c = pbo.tile([P, nwin * HU], f32, tag="osb")
                    osb3 = osb_c[:].rearrange("p (i c) -> p i c", c=HU)
                    off = nwin
                    for i in range(nwin):
                        w = w0 + i
                        k = K[w]
                        if k == 0:
                            nc.gpsimd.memset(osb3[:, i, :], 0.0)
                            continue
                        # receiver s2 for both halves -> selected s2 [P, 4]
                        s2m = pbw.tile([P, PAIRC], f16, tag="s2m")
                        nc.vector.tensor_tensor(
                            out=s2m[:], in0=hs3[:, i, :],
                            in1=aa_sb[:, PAIRC:2 * PAIRC],
                            op=mybir.AluOpType.mult)
                        s2b = pbw.tile([P, 8], f32, tag="s2b")
                        nc.vector.tensor_reduce(
                            out=s2b[:],
                            in_=s2m[:].rearrange("p (g u) -> p g u", u=UNITS),
                            axis=mybir.AxisListType.X, op=mybir.AluOpType.add)
                        s2s = pbw.tile([P, 8], f32, tag="s2s")
                        nc.vector.tensor_tensor(
                            out=s2s[:], in0=s2b[:], in1=m3[:, i, :],
                            op=mybir.AluOpType.mult)
                        s2 = pbw.tile([P, HEADS], f32, tag="s2")
                        nc.vector.tensor_tensor(
                            out=s2[:], in0=s2s[:, 0:4], in1=s2s[:, 4:8],
                            op=mybir.AluOpType.add)

                        # sender s1 for both halves [P, k*8]
                        s1m = pbw.tile([P, k * PAIRC], f16, tag="s1m")
                        nc.vector.tensor_tensor(
                            out=s1m[:].rearrange("p (j c) -> p j c", c=PAIRC),
                            in0=hs3[:, off:off + k, :],
                            in1=aa_sb[:, 0:PAIRC].unsqueeze(1).broadcast_to(
                                [P, k, PAIRC]),
                            op=mybir.AluOpType.mult)
                        s1b = pbw.tile([P, k * 8], f16, tag="s1b")
                        with nc.allow_low_precision(
                                reason="fp16 logits OK at 2e-2 tol"):
                            nc.vector.tensor_reduce(
                                out=s1b[:],
                                in_=s1m[:].rearrange("p (a u) -> p a u",
                                                     u=UNITS),
                                axis=mybir.AxisListType.X,
                                op=mybir.AluOpType.add)

                        # logits: s1 + s2(recv) + mask, leaky, exp
                        lg = pbw.tile([P, k * 8], f16, tag="lg")
                        nc.vector.tensor_tensor(
                            out=lg[:].rearrange("p (j t h) -> p j t h", t=2,
                                                h=HEADS),
                            in0=s1b[:].rearrange("p (j t h) -> p j t h", t=2,
                                                 h=HEADS),
                            in1=s2[:].unsqueeze(1).unsqueeze(2).broadcast_to(
                                [P, k, 2, HEADS]),
                            op=mybir.AluOpType.add)
                        lgm = pbw.tile([P, k * 8], f16, tag="lgm")
                        nc.vector.tensor_tensor(
                            out=lgm[:],
                            in0=lg[:],
                            in1=m3[:, off:off + k, :].rearrange(
                                "p j q -> p (j q)"),
                            op=mybir.AluOpType.add)
                        neg = pbw.tile([P, k * 8], f16, tag="neg")
                        nc.vector.tensor_scalar(
                            out=neg[:], in0=lgm[:], scalar1=0.0,
                            scalar2=LEAKY_ALPHA, op0=mybir.AluOpType.min,
                            op1=mybir.AluOpType.mult)
                        lr = pbw.tile([P, k * 8], f16, tag="lr")
                        nc.vector.scalar_tensor_tensor(
                            out=lr[:], in0=lgm[:], scalar=0.0, in1=neg[:],
                            op0=mybir.AluOpType.max, op1=mybir.AluOpType.add)
                        expo = pbw.tile([P, k * 8], f16, tag="expo")
                        nc.scalar.activation(
                            out=expo[:], in_=lr[:],
                            func=mybir.ActivationFunctionType.Exp,
                            bias=cbias[:])

                        # weighted sender features (both halves)
                        rhs = pbw.tile([P, k * PAIRC], f16, tag="rhs")
                        nc.vector.tensor_tensor(
                            out=rhs[:].rearrange("p (j g u) -> p j g u",
                                                 g=8, u=UNITS),
                            in0=hs3[:, off:off + k, :].rearrange(
                                "p j (g u) -> p j g u", u=UNITS),
                            in1=expo[:].rearrange("p (j g) -> p j g",
                                                  g=8).unsqueeze(
                                3).broadcast_to([P, k, 8, UNITS]),
                            op=mybir.AluOpType.mult)

                        den = pbw.tile([P, 8], f32, tag="den")
                        nc.vector.tensor_reduce(
                            out=den[:],
                            in_=expo[:].rearrange("p (j q) -> p q j", q=8),
                            axis=mybir.AxisListType.X, op=mybir.AluOpType.add)
                        num = pbw.tile([P, PAIRC], f32, tag="num")
                        nc.vector.tensor_reduce(
                            out=num[:],
                            in_=rhs[:].rearrange("p (j c) -> p c j", c=PAIRC),
                            axis=mybir.AxisListType.X, op=mybir.AluOpType.add)

                        den4 = pbw.tile([P, HEADS], f32, tag="den4")
                        nc.vector.tensor_tensor(
                            out=den4[:], in0=den[:, 0:4], in1=den[:, 4:8],
                            op=mybir.AluOpType.add)
                        num64 = pbw.tile([P, HU], f32, tag="num64")
                        nc.vector.tensor_tensor(
                            out=num64[:], in0=num[:, 0:HU], in1=num[:, HU:],
                            op=mybir.AluOpType.add)
                        den4e = pbw.tile([P, HEADS], f32, tag="den4e")
                        nc.vector.tensor_scalar_add(
                            out=den4e[:], in0=den4[:], scalar1=1e-30)
                        rec = pbw.tile([P, HEADS], f32, tag="rec")
                        nc.vector.reciprocal(out=rec[:], in_=den4e[:])
                        nc.vector.tensor_tensor(
                            out=osb3[:, i, :].rearrange("p (h u) -> p h u",
                                                        u=UNITS),
                            in0=num64[:].rearrange("p (h u) -> p h u",
                                                   u=UNITS),
                            in1=rec[:].unsqueeze(2).broadcast_to(
                                [P, HEADS, UNITS]),
                            op=mybir.AluOpType.mult)
                        off += k
                    nc.sync.dma_start(
                        out=out_d[w0 * P:(w0 + nwin) * P, :].rearrange(
                            "(i p) c -> p i c", p=P),
                        in_=osb3)

    nc.compile()
    return nc


def _run(nc, per_core, n_cores):
    from concourse import bass_utils

    want_trace = bool(os.environ.get("GAT_TRACE"))
    res = bass_utils.run_bass_kernel_spmd(
        nc, per_core, core_ids=list(range(n_cores)), trace=want_trace)
    return res


def _unshard(host, results, n_cores):
    n_nodes = host["n_nodes"]
    out = np.zeros((n_nodes, HU), dtype=np.float32)
    for c in range(n_cores):
        staged = results[c]["staged"]  # [nw*128, 64]
        wn = host["win_nodes"][c]      # [nw, 128]
        valid = wn >= 0
        out[wn[valid]] = staged.reshape(wn.shape[0], P, HU)[valid]
    return out


def kernel(x, edge_index, W, att_w1, att_w2, n_cores=8, _return_results=False):
    x = np.asarray(x)
    edge_index = np.asarray(edge_index)
    W = np.asarray(W).astype(np.float32)
    att_w1 = np.asarray(att_w1).astype(np.float32)
    att_w2 = np.asarray(att_w2).astype(np.float32)

    host, per_core = _build_host_data(x, edge_index, W, att_w1, att_w2, n_cores)
    nc = _build_bass(host["plan"])
    res = _run(nc, per_core, n_cores)
    out = _unshard(host, res.results, n_cores)
    if _return_results:
        return out, res
    return out
